# revision 1
# baseline (speedup 1.0000x reference)
"""CTC loss (Keras ctc_batch_cost semantics) on 8 Trainium2 NeuronCores.

Strategy
--------
Data-parallel over batch: core q handles examples [32q, 32q+32).

Math: the CTC alpha recursion is run in *linear probability space* with
per-block (8 blocks x 16 label positions) floating scales ("block-float"),
which provides the ~1e-100 dynamic range the reference's log-space
recursion implicitly has, while keeping the per-step work to a handful of
elementwise ops (no transcendentals in the loop).

State (per example), derived from the de-interleaved (blank/label) lattice:
    g[j]  = La[j] + Dt[j]            (j = 0..127 label positions)
    D[i]  = Bl[i] + La[i-1]          (i = 0..128 blank positions)
with update (one time step, p = y_pred + eps):
    h  = g * pl      ( = new La )
    ht = h * ksm     (skip mask folded in)
    D' = D*pb + shift1(h)
    tm = D*pb + shift1(ht)
    g' = h + tm
Time is split forward/backward: rows 0-31 of each on-chip tile run alpha
forward over t = 0..511, rows 32-63 run the suffix recursion ("beta-hat")
backward over t = 1023..512 in reversed label coordinates, which has the
identical banded structure.  After 512 steps both meet and
    P = sum_s alpha[511, s] * beta_hat[511, s]
is combined in log space on-device.  Per-step symbol probabilities are
gathered on TensorE as one-hot matmuls (Sel built on-device from the
labels via iota-compare) and flattened into per-step-sliceable chunks.

The state tiles use a "gapped" layout [64, 8*17+1]: each 16-wide block is
preceded by a crossing slot holding the scale-adjusted neighbour value, so
the shift-by-one reads cross scale boundaries through plain strided APs.

Everything in kernel() besides the device run is input marshalling:
sharding, the (y_pred + eps) transpose into c-major halves, and the tiny
label-derived constant tensors (labels+blank list, skip masks, iota).
"""

import os
import sys
from contextlib import ExitStack

import numpy as np

# ---------------- problem constants (hardcoded per the task spec) -------------
B, T, C, L = 256, 1024, 256, 128
NCORES = 8
NE = B // NCORES          # examples per core (32)
VR = 2 * NE               # virtual rows: fwd + bwd (64)
TH = T // 2               # sequential steps per half (512)
NB = 8                    # scale blocks
BW = 16                   # block width (label positions)
SW = BW + 1               # gapped slot width
WS = NB * SW + 1          # state tile width (137); col 136 = D[128] tail
NT = 64                   # time-block (chunk) size
NCHUNK = TH // NT         # 8
RB = 32                   # rebalance interval
EPS = 1e-7
CAPM = float(np.exp(-30.0))   # e^-CAP cone slope
FLOORV = 1e-35
TINY = 1e-37
NEG = -1e18

_TRN_REPO = "/opt/trn_rl_repo"


def _ensure_path():
    if _TRN_REPO not in sys.path:
        sys.path.insert(0, _TRN_REPO)


# ----------------------------- device kernel ---------------------------------

def build_nc(debug_taps=False):
    """Build and compile the Bass module (same NEFF for all 8 cores)."""
    _ensure_path()
    import concourse.bass as bass
    import concourse.mybir as mybir
    import concourse.tile as tile
    from concourse import bacc

    f32 = mybir.dt.float32
    Alu = mybir.AluOpType
    Act = mybir.ActivationFunctionType

    nc = bacc.Bacc(
        "TRN2", target_bir_lowering=False, debug=False, num_devices=NCORES
    )

    xpf_d = nc.dram_tensor("xpf", [NE, C, TH], f32, kind="ExternalInput").ap()
    xpb_d = nc.dram_tensor("xpb", [NE, C, TH], f32, kind="ExternalInput").ap()
    labext_d = nc.dram_tensor("labext", [128, VR, 129], f32, kind="ExternalInput").ap()
    scr_d = nc.dram_tensor("scr", [2, NT, NT * 129], f32).ap()
    ksm_d = nc.dram_tensor("ksm", [VR, L], f32, kind="ExternalInput").ap()
    iota2_d = nc.dram_tensor("iota2", [128, 2], f32, kind="ExternalInput").ap()
    loss_d = nc.dram_tensor("loss", [NE, 1], f32, kind="ExternalOutput").ap()
    if debug_taps:
        dbg_plc = nc.dram_tensor("dbg_plc", [VR, NT * 129], f32, kind="ExternalOutput").ap()
        dbg_g = nc.dram_tensor("dbg_g", [VR, WS], f32, kind="ExternalOutput").ap()
        dbg_D = nc.dram_tensor("dbg_D", [VR, WS], f32, kind="ExternalOutput").ap()
        dbg_lZ = nc.dram_tensor("dbg_lZ", [VR, NB], f32, kind="ExternalOutput").ap()
        dbg_LL = nc.dram_tensor("dbg_LL", [VR, L], f32, kind="ExternalOutput").ap()
        dbg_LLb = nc.dram_tensor("dbg_LLb", [NE, L], f32, kind="ExternalOutput").ap()
        dbg_sel = nc.dram_tensor("dbg_sel", [128, 129], f32, kind="ExternalOutput").ap()
        dbg_lZh = nc.dram_tensor("dbg_lZh", [TH // RB, VR, NB], f32, kind="ExternalOutput").ap()
        dbg_bmh = nc.dram_tensor("dbg_bmh", [TH // RB, VR, NB], f32, kind="ExternalOutput").ap()

    def pay(t):   # payload [VR, 8, 16]: block k cols 17k+1 .. 17k+16
        return t[:, 1:WS].rearrange("p (b s) -> p b s", s=SW)[:, :, 0:BW]

    def sl015(t):  # shifted source [VR, 8, 16]: slots 0..15 per block
        return t[:, 0:NB * SW].rearrange("p (b s) -> p b s", s=SW)[:, :, 0:BW]

    def s16(t):   # crossing sources: slot 16 of blocks 0..6 -> [VR, 7, 1]
        return t[:, 0:NB * SW].rearrange("p (b s) -> p b s", s=SW)[:, 0:7, BW:BW + 1]

    def s0(t):    # crossing dests: slot 0 of blocks 1..7 -> [VR, 7, 1]
        return t[:, 0:NB * SW].rearrange("p (b s) -> p b s", s=SW)[:, 1:8, 0:1]

    def tail(t):
        return t[:, WS - 1:WS]

    def bpay(t, k):  # one block's payload, contiguous [VR, 16]
        return t[:, SW * k + 1: SW * k + 1 + BW]

    with tile.TileContext(nc) as tc, ExitStack() as ctx:
        const_p = ctx.enter_context(tc.tile_pool(name="const", bufs=1))
        state_p = ctx.enter_context(tc.tile_pool(name="state", bufs=1))
        sel_p = ctx.enter_context(tc.tile_pool(name="sel", bufs=1))
        chunk_p = ctx.enter_context(tc.tile_pool(name="chunk", bufs=2))
        xt_p = ctx.enter_context(tc.tile_pool(name="xt", bufs=1))
        ps_p = ctx.enter_context(
            tc.tile_pool(name="ps", bufs=4, space=bass.MemorySpace.PSUM)
        )
        evb_p = ctx.enter_context(tc.tile_pool(name="evb", bufs=1))
        lab_p = ctx.enter_context(tc.tile_pool(name="lab", bufs=2))
        ep_p = ctx.enter_context(tc.tile_pool(name="ep", bufs=1))

        V, S, G = nc.vector, nc.scalar, nc.gpsimd

        # ---- constants
        ksm_t = const_p.tile([VR, L], f32, tag="ksm")
        nc.sync.dma_start(ksm_t[:, :], ksm_d[:, :])
        iota_t = const_p.tile([128, 2], f32, tag="iota")
        nc.sync.dma_start(iota_t[:, :], iota2_d[:, :])
        # ---- Sel matrices: [128c, 129] per (virtual row, c-chunk)
        sels = []
        for v in range(VR):
            labrep = lab_p.tile([128, 129], f32, tag="labrep")
            nc.sync.dma_start(labrep[:, :], labext_d[:, v, :])
            row = []
            for cc in range(2):
                st = sel_p.tile([128, 129], f32, tag=f"sel_{v}_{cc}")
                V.tensor_single_scalar(
                    st[:, :], labrep[:, :], iota_t[:, cc:cc + 1], Alu.is_equal
                )
                row.append(st)
            sels.append(row)
        if debug_taps:
            nc.sync.dma_start(dbg_sel[:, :], sels[0][0][:, :])

        # ---- state
        def zstate(tag):
            t = state_p.tile([VR, WS], f32, tag=tag)
            V.memset(t[:, :], 0.0)
            return t

        gA, gB = zstate("gA"), zstate("gB")
        DA, DB = zstate("DA"), zstate("DB")
        hA, hB = zstate("hA"), zstate("hB")
        htA, htB = zstate("htA"), zstate("htB")
        tmA, tmB = zstate("tmA"), zstate("tmB")
        utA, utB = zstate("utA"), zstate("utB")
        V.memset(gA[:, 1:2], 1.0)   # g[j=0] = 1
        V.memset(DA[:, 1:2], 1.0)   # D[i=0] = 1

        lZ = state_p.tile([VR, NB], f32, tag="lZ")
        V.memset(lZ[:, :], 0.0)
        rv = state_p.tile([VR, 7], f32, tag="rv")
        V.memset(rv[:, :], 1.0)
        bm = state_p.tile([VR, NB], f32, tag="bm")
        eqt = state_p.tile([VR, NB], f32, tag="eqt")
        rct = state_p.tile([VR, 7], f32, tag="rct")
        rvi = state_p.tile([VR, 7], f32, tag="rvi")
        invb = state_p.tile([VR, NB], f32, tag="invb")
        lbt = state_p.tile([VR, NB], f32, tag="lbt")
        bml = state_p.tile([VR, NB], f32, tag="bml")
        dlt = state_p.tile([VR, 7], f32, tag="dlt")
        i32 = mybir.dt.int32
        sln_i = state_p.tile([VR, 129], i32, tag="sln_i")
        sln_m = state_p.tile([VR, 129], i32, tag="sln_m")
        sln_e = state_p.tile([VR, 129], f32, tag="sln_e")
        sln_l = state_p.tile([VR, 129], f32, tag="sln_l")

        def safe_ln(dst_ap, src_ap, n, rows=slice(0, VR)):
            """dst = ln(src) via exponent extraction; HW Ln table is only
            accurate on ~[1e-10, 2^64], mantissa lives in [1, 2)."""
            ii = sln_i[rows, 0:n]
            mm = sln_m[rows, 0:n]
            ee = sln_e[rows, 0:n]
            ll = sln_l[rows, 0:n]
            V.tensor_single_scalar(ii, src_ap.bitcast(i32), 23, Alu.arith_shift_right)
            V.tensor_single_scalar(ii, ii, 127, Alu.subtract)
            V.tensor_single_scalar(mm, src_ap.bitcast(i32), 0x007FFFFF, Alu.bitwise_and)
            V.tensor_single_scalar(mm, mm, 0x3F800000, Alu.bitwise_or)
            S.activation(ll, mm.bitcast(f32), Act.Ln)
            V.tensor_copy(ee, ii)
            V.scalar_tensor_tensor(
                dst_ap, ee, 0.6931471805599453, ll, Alu.mult, Alu.add
            )

        cur = {"g": gA, "D": DA}
        new = {"g": gB, "D": DB}
        hcur, hnew = hA, hB
        htcur, htnew = htA, htB
        tmcur, tmnew = tmA, tmB
        utcur, utnew = utA, utB

        def rebalance(Dn, gn, extra_scaled=()):
            V.tensor_reduce(bm[:, :], pay(Dn), axis=mybir.AxisListType.X, op=Alu.max)
            V.tensor_tensor(bm[:, 7:8], bm[:, 7:8], tail(Dn), Alu.max)
            V.tensor_single_scalar(eqt[:, :], bm[:, :], 0.0, Alu.is_equal)
            V.tensor_tensor(bm[:, :], bm[:, :], eqt[:, :], Alu.add)
            V.tensor_single_scalar(rct[:, :], rv[:, :], CAPM, Alu.mult)
            for k in range(1, NB):
                V.scalar_tensor_tensor(
                    bm[:, k:k + 1], bm[:, k - 1:k], rct[:, k - 1:k],
                    bm[:, k:k + 1], Alu.mult, Alu.max,
                )
            V.reciprocal(rvi[:, :], rv[:, :])
            V.tensor_single_scalar(rci := rct, rvi[:, :], CAPM, Alu.mult)
            for k in range(NB - 2, -1, -1):
                V.scalar_tensor_tensor(
                    bm[:, k:k + 1], bm[:, k + 1:k + 2], rci[:, k:k + 1],
                    bm[:, k:k + 1], Alu.mult, Alu.max,
                )
            V.tensor_single_scalar(bm[:, :], bm[:, :], FLOORV, Alu.max)
            V.reciprocal(invb[:, :], bm[:, :])
            for k in range(NB):
                V.tensor_single_scalar(
                    bpay(Dn, k), bpay(Dn, k), invb[:, k:k + 1], Alu.mult
                )
                V.tensor_single_scalar(
                    bpay(gn, k), bpay(gn, k), invb[:, k:k + 1], Alu.mult
                )
                for ex in extra_scaled:
                    V.tensor_single_scalar(
                        bpay(ex, k), bpay(ex, k), invb[:, k:k + 1], Alu.mult
                    )
            V.tensor_single_scalar(tail(Dn), tail(Dn), invb[:, 7:8], Alu.mult)
            for ex in extra_scaled:
                V.tensor_single_scalar(tail(ex), tail(ex), invb[:, 7:8], Alu.mult)
            safe_ln(lbt[:, :], bm[:, :], NB)
            V.tensor_tensor(lZ[:, :], lZ[:, :], lbt[:, :], Alu.add)
            # multiplicative rv update (no Exp: avoids ACT table-set thrash).
            # order (rv*bm)*inv keeps intermediates < 3.4e38 for CAP=30.
            V.tensor_tensor(dlt[:, :], rv[:, :], bm[:, 0:7], Alu.mult)
            V.tensor_tensor(rv[:, :], dlt[:, :], invb[:, 1:8], Alu.mult)

        # ---- main loop
        for blk in range(NCHUNK):
            # two big X loads per chunk (one per direction): [c, cc, e, t]
            xts = []
            for d, srcd in enumerate((xpf_d, xpb_d)):
                xb = xt_p.tile([128, 2 * NE, NT], f32, tag=f"xb{d}")
                nc.sync.dma_start(
                    xb[:, :, :],
                    srcd.rearrange("e (cc c) t -> c (e cc) t", cc=2)[
                        :, :, blk * NT:(blk + 1) * NT
                    ],
                )
                xts.append(xb)
            scrb = scr_d[blk % 2].rearrange("m (v s) -> m v s", s=129)
            for half in range(2):
                evb = evb_p.tile([NT, NE * 129], f32, tag="evb")
                for vq in range(NE):
                    v = half * NE + vq
                    e = v % NE
                    xb = xts[half]
                    ps = ps_p.tile([NT, 129], f32, tag="gps")
                    nc.tensor.matmul(
                        ps[:, :], xb[:, 2 * e, :], sels[v][0][:, :],
                        start=True, stop=False,
                    )
                    nc.tensor.matmul(
                        ps[:, :], xb[:, 2 * e + 1, :], sels[v][1][:, :],
                        start=False, stop=True,
                    )
                    S.copy(evb[:, vq * 129:(vq + 1) * 129], ps[:, :])
                # bounce to DRAM: [t-part, v-major] -> flat
                nc.sync.dma_start(
                    scrb[:, half * NE:(half + 1) * NE, :], evb[:, :]
                )
            plc = chunk_p.tile([VR, NT * 129], f32, tag="plc")
            nc.sync.dma_start(
                plc[:, :],
                scr_d[blk % 2].rearrange("m (v s) -> v m s", s=129),
            )
            if debug_taps and blk == 0:
                nc.sync.dma_start(dbg_plc[:, :], plc[:, :])

            for ml in range(NT):
                m = blk * NT + ml
                pls = plc[:, ml * 129: ml * 129 + 128]
                pbs = plc[:, ml * 129 + 128: ml * 129 + 129]
                g0, D0 = cur["g"], cur["D"]
                g1, D1 = new["g"], new["D"]

                pbs_b = pbs.broadcast_to([VR, WS])
                G.tensor_tensor(pay(hnew), pay(g0), pls, Alu.mult)
                G.tensor_tensor(s0(hnew), s16(hnew), rv[:, :], Alu.mult)
                G.tensor_tensor(pay(htnew), pay(hnew), ksm_t[:, :], Alu.mult)
                G.tensor_tensor(s0(htnew), s16(htnew), rv[:, :], Alu.mult)
                G.tensor_tensor(utnew[:, :], D0[:, :], pbs_b, Alu.mult)
                G.tensor_tensor(
                    pay(tmnew), pay(utnew), sl015(htnew), Alu.add
                )
                G.tensor_tensor(
                    pay(D1), pay(utnew), sl015(hnew), Alu.add
                )
                G.tensor_tensor(
                    tail(D1), tail(utnew), hnew[:, 135:136], Alu.add
                )
                G.tensor_tensor(pay(g1), pay(hnew), pay(tmnew), Alu.add)

                if (m + 1) % RB == 0:
                    extra = (hnew, utnew) if m == TH - 1 else ()
                    rebalance(D1, g1, extra_scaled=extra)
                    if debug_taps:
                        r = (m + 1) // RB - 1
                        nc.sync.dma_start(dbg_lZh[r], lZ[:, :])
                        nc.sync.dma_start(dbg_bmh[r], bm[:, :])

                cur, new = new, cur
                hcur, hnew = hnew, hcur
                htcur, htnew = htnew, htcur
                tmcur, tmnew = tmnew, tmcur
                utcur, utnew = utnew, utcur
                if debug_taps and m == NT - 1:
                    nc.sync.dma_start(dbg_g[:, :], cur["g"][:, :])
                    nc.sync.dma_start(dbg_D[:, :], cur["D"][:, :])

        # ---- epilogue: log-space combine
        # final tensors: fwd rows: aL = hcur, aB = uT; bwd rows: bL = cur g, bB = cur D
        gF, DF = cur["g"], cur["D"]
        hF = hcur
        uT = utcur
        LL = ep_p.tile([VR, L], f32, tag="LL")
        LBt = ep_p.tile([VR, 129], f32, tag="LB")
        cl1 = ep_p.tile([VR, L], f32, tag="cl1")
        eq1 = ep_p.tile([VR, L], f32, tag="eq1")
        ln1 = ep_p.tile([VR, L], f32, tag="ln1")
        cl2 = ep_p.tile([VR, 129], f32, tag="cl2")
        eq2 = ep_p.tile([VR, 129], f32, tag="eq2")
        ln2 = ep_p.tile([VR, 129], f32, tag="ln2")

        V.tensor_single_scalar(cl1[0:NE, :], pay(hF)[0:NE], TINY, Alu.max)
        V.tensor_single_scalar(cl1[NE:VR, :], pay(gF)[NE:VR], TINY, Alu.max)
        V.tensor_single_scalar(eq1[0:NE, :], pay(hF)[0:NE], 0.0, Alu.is_equal)
        V.tensor_single_scalar(eq1[NE:VR, :], pay(gF)[NE:VR], 0.0, Alu.is_equal)
        safe_ln(ln1[:, :], cl1[:, :], L)
        V.scalar_tensor_tensor(LL[:, :], eq1[:, :], NEG, ln1[:, :], Alu.mult, Alu.add)

        V.tensor_single_scalar(cl2[0:NE, 0:L], pay(uT)[0:NE], TINY, Alu.max)
        V.tensor_single_scalar(cl2[0:NE, L:129], tail(uT)[0:NE], TINY, Alu.max)
        V.tensor_single_scalar(cl2[NE:VR, 0:L], pay(DF)[NE:VR], TINY, Alu.max)
        V.tensor_single_scalar(cl2[NE:VR, L:129], tail(DF)[NE:VR], TINY, Alu.max)
        V.tensor_single_scalar(eq2[0:NE, 0:L], pay(uT)[0:NE], 0.0, Alu.is_equal)
        V.tensor_single_scalar(eq2[0:NE, L:129], tail(uT)[0:NE], 0.0, Alu.is_equal)
        V.tensor_single_scalar(eq2[NE:VR, 0:L], pay(DF)[NE:VR], 0.0, Alu.is_equal)
        V.tensor_single_scalar(eq2[NE:VR, L:129], tail(DF)[NE:VR], 0.0, Alu.is_equal)
        safe_ln(ln2[:, :], cl2[:, :], 129)
        V.scalar_tensor_tensor(LBt[:, :], eq2[:, :], NEG, ln2[:, :], Alu.mult, Alu.add)

        for k in range(NB):
            V.tensor_single_scalar(
                LL[:, BW * k:BW * k + BW], LL[:, BW * k:BW * k + BW],
                lZ[:, k:k + 1], Alu.add,
            )
            hi = BW * k + BW + (1 if k == NB - 1 else 0)
            V.tensor_single_scalar(
                LBt[:, BW * k:hi], LBt[:, BW * k:hi], lZ[:, k:k + 1], Alu.add
            )

        LLb = ep_p.tile([NE, L], f32, tag="LLb")
        LBb = ep_p.tile([NE, 129], f32, tag="LBb")
        nc.sync.dma_start(LLb[:, :], LL[NE:VR, ::-1])
        nc.sync.dma_start(LBb[:, :], LBt[NE:VR, ::-1])
        if debug_taps:
            nc.sync.dma_start(dbg_lZ[:, :], lZ[:, :])
            nc.sync.dma_start(dbg_LL[:, :], LL[:, :])
            nc.sync.dma_start(dbg_LLb[:, :], LLb[:, :])

        lPL = ep_p.tile([NE, L], f32, tag="lPL")
        lPB = ep_p.tile([NE, 129], f32, tag="lPB")
        V.tensor_tensor(lPL[:, :], LL[0:NE, :], LLb[:, :], Alu.add)
        V.tensor_tensor(lPB[:, :], LBt[0:NE, :], LBb[:, :], Alu.add)

        m1 = ep_p.tile([NE, 1], f32, tag="m1")
        m2 = ep_p.tile([NE, 1], f32, tag="m2")
        V.tensor_reduce(m1[:, :], lPL[:, :], axis=mybir.AxisListType.X, op=Alu.max)
        V.tensor_reduce(m2[:, :], lPB[:, :], axis=mybir.AxisListType.X, op=Alu.max)
        V.tensor_tensor(m1[:, :], m1[:, :], m2[:, :], Alu.max)
        sm1 = ep_p.tile([NE, L], f32, tag="sm1")
        sm2 = ep_p.tile([NE, 129], f32, tag="sm2")
        V.tensor_single_scalar(sm1[:, :], lPL[:, :], m1[:, 0:1], Alu.subtract)
        V.tensor_single_scalar(sm2[:, :], lPB[:, :], m1[:, 0:1], Alu.subtract)
        e1 = ep_p.tile([NE, L], f32, tag="e1")
        e2 = ep_p.tile([NE, 129], f32, tag="e2")
        s1 = ep_p.tile([NE, 1], f32, tag="s1")
        s2 = ep_p.tile([NE, 1], f32, tag="s2")
        S.activation(e1[:, :], sm1[:, :], Act.Exp, accum_out=s1[:, :])
        S.activation(e2[:, :], sm2[:, :], Act.Exp, accum_out=s2[:, :])
        V.tensor_tensor(s1[:, :], s1[:, :], s2[:, :], Alu.add)
        lsum = ep_p.tile([NE, 1], f32, tag="lsum")
        S.activation(lsum[:, :], s1[:, :], Act.Ln)
        lossT = ep_p.tile([NE, 1], f32, tag="lossT")
        V.tensor_tensor(lossT[:, :], m1[:, :], lsum[:, :], Alu.add)
        V.tensor_single_scalar(lossT[:, :], lossT[:, :], -1.0, Alu.mult)
        nc.sync.dma_start(loss_d[:, :], lossT[:, :])

    nc.compile()
    return nc


def build_stub_nc():
    """Same I/O signature as build_nc but ~zero device work (for dispatch-
    overhead calibration in timing.py)."""
    _ensure_path()
    import concourse.bass as bass
    import concourse.mybir as mybir
    import concourse.tile as tile
    from concourse import bacc
    from contextlib import ExitStack

    f32 = mybir.dt.float32
    nc = bacc.Bacc("TRN2", target_bir_lowering=False, debug=False, num_devices=NCORES)
    nc.dram_tensor("xpf", [NE, C, TH], f32, kind="ExternalInput").ap()
    nc.dram_tensor("xpb", [NE, C, TH], f32, kind="ExternalInput").ap()
    labext_d = nc.dram_tensor("labext", [128, VR, 129], f32, kind="ExternalInput").ap()
    nc.dram_tensor("ksm", [VR, L], f32, kind="ExternalInput").ap()
    nc.dram_tensor("iota2", [128, 2], f32, kind="ExternalInput").ap()
    loss_d = nc.dram_tensor("loss", [NE, 1], f32, kind="ExternalOutput").ap()
    with tile.TileContext(nc) as tc, ExitStack() as ctx:
        p = ctx.enter_context(tc.tile_pool(name="p", bufs=1))
        t = p.tile([NE, 1], f32, tag="t")
        nc.sync.dma_start(t[:, :], labext_d[0, 0:NE, 0:1])
        nc.sync.dma_start(loss_d[:, :], t[:, :])
    nc.compile()
    return nc


# ------------------------------- host side ------------------------------------

def make_inputs(y_true, y_pred):
    """Build the 8 per-core input maps."""
    y_true = np.asarray(y_true)
    y_pred = np.asarray(y_pred, dtype=np.float32)
    p = y_pred + np.float32(EPS)                       # [B, T, C]

    iota2 = np.empty((128, 2), np.float32)
    iota2[:, 0] = np.arange(128, dtype=np.float32)
    iota2[:, 1] = np.arange(128, 256, dtype=np.float32)

    in_maps = []
    for q in range(NCORES):
        sl = slice(q * NE, (q + 1) * NE)
        pq = p[sl]                                      # [NE, T, C]
        xpf = np.ascontiguousarray(pq[:, :TH, :].transpose(0, 2, 1))
        xpb = np.ascontiguousarray(pq[:, TH:, :][:, ::-1, :].transpose(0, 2, 1))
        lab = y_true[sl].astype(np.int64)               # [NE, L]
        k = np.ones((NE, L), np.float32)
        k[:, 1:] = (lab[:, 1:] != lab[:, :-1]).astype(np.float32)
        ks = np.ones((NE, L), np.float32)
        ks[:, :-1] = k[:, 1:]
        ksb = k[:, ::-1].copy()
        ksm = np.concatenate([ks, ksb], axis=0)         # [VR, L]
        labext = np.empty((VR, 129), np.float32)
        labext[:NE, :L] = lab
        labext[NE:, :L] = lab[:, ::-1]
        labext[:, L] = C - 1
        labrep = np.broadcast_to(labext[None, :, :], (128, VR, 129)).copy()
        in_maps.append({
            "xpf": xpf,
            "xpb": xpb,
            "labext": labrep,
            "ksm": ksm,
            "iota2": iota2,
        })
    return in_maps


_NC_CACHE = {}


def _get_nc():
    if "nc" not in _NC_CACHE:
        _NC_CACHE["nc"] = build_nc()
    return _NC_CACHE["nc"]


def kernel(y_true, y_pred):
    _ensure_path()
    from concourse.bass_utils import run_bass_kernel_spmd

    nc = _get_nc()
    in_maps = make_inputs(y_true, y_pred)
    res = run_bass_kernel_spmd(nc, in_maps, core_ids=list(range(NCORES)))
    loss = np.concatenate([r["loss"] for r in res.results], axis=0)
    return loss.astype(np.float32)


if __name__ == "__main__":
    # smoke: build only
    nc = build_nc()
    print("built + compiled OK")



# revision 3
# speedup vs baseline: 2.4864x; 2.4864x over previous
"""CTC loss (Keras ctc_batch_cost semantics) on 8 Trainium2 NeuronCores.

Strategy
--------
Data-parallel over batch: core q handles examples [32q, 32q+32).

The CTC alpha recursion runs in *linear probability space* with a single
per-row floating scale plus a static exponential position equilibration
(state stored as G[j] = g[j]*e^{-c*j}, D[i] = d[i]*e^{-c*(i-1)}, c=3.5),
which flattens the huge position tilt of the lattice (~3.5 nats/position)
so one scale per row suffices for fp32/bf16 exponent range.  The
equilibration constants fold into the ksm mask, an extra gathered blank
column (pb*e^{-c}), the D init value, and two scalar epilogue constants —
zero extra per-step work.

Time is split forward/backward (meet in the middle): rows 0-31 of each
on-chip tile run alpha forward over t = 0..511, rows 32-63 run the suffix
recursion backward over t = 1023..512 in reversed label coordinates.

Per time step the whole update is 5 DVE (vector engine) instructions in
bf16 (fastest engine; scalar_tensor_tensor fuses the D*pb+shift adds):
    h  = g0 * pl                       TT   [64,128]
    t1 = (D0 * pb2) + h                STT  [64,128]
    D1 = (D0 * pb ) + shift1(h)        STT  [64,129]
    kh = h * ksm~                      TT   [64,128]
    g1 = t1 + shift1(kh)               STT->TT [64,128]
Shifts are plain contiguous APs via a permanent zero guard column in the
h/kh tiles.  Rescale by the D-row-max every 32 steps (4 more DVE ops).

Per-step symbol probabilities are gathered on TensorE as one-hot matmuls
in bf16 (Sel matrices prebuilt on host, incl. the e^{-c} blank column),
bounced through DRAM to transpose [time-major] -> [row-major], all in
bf16.  ScalarE does the PSUM->SBUF copies.  Everything overlaps the
serial 512-step DVE chain.
"""

import sys
from contextlib import ExitStack

import numpy as np
import ml_dtypes

# ---------------- problem constants (hardcoded per the task spec) -------------
B, T, C, L = 256, 1024, 256, 128
NCORES = 8
NE = B // NCORES          # examples per core (32)
VR = 2 * NE               # virtual rows: fwd + bwd (64)
TH = T // 2               # sequential steps per half (512)
NT = 128                  # time-block (chunk) size
NCHUNK = TH // NT         # 4
SW = 130                  # gathered columns: 128 labels + pb + pb*e^-c
RB = 32                   # rescale interval
NR = TH // RB             # number of rescales (16)
EPS = 1e-7
CEQ = 3.5                 # equilibration nats/position
TINY = 1e-37
NEG = -1e18

_TRN_REPO = "/opt/trn_rl_repo"
BF16NP = ml_dtypes.bfloat16


def _ensure_path():
    if _TRN_REPO not in sys.path:
        sys.path.insert(0, _TRN_REPO)


# ----------------------------- device kernel ---------------------------------

def build_nc():
    """Build and compile the Bass module (same NEFF for all 8 cores)."""
    _ensure_path()
    import concourse.bass as bass
    import concourse.mybir as mybir
    import concourse.tile as tile
    from concourse import bacc

    f32 = mybir.dt.float32
    bf16 = mybir.dt.bfloat16
    i32 = mybir.dt.int32
    Alu = mybir.AluOpType
    Act = mybir.ActivationFunctionType

    nc = bacc.Bacc(
        "TRN2", target_bir_lowering=False, debug=False, num_devices=NCORES
    )

    xpf_d = nc.dram_tensor("xpf", [NE, C, TH], bf16, kind="ExternalInput").ap()
    xpb_d = nc.dram_tensor("xpb", [NE, C, TH], bf16, kind="ExternalInput").ap()
    sel_d = nc.dram_tensor("sel", [128, 2 * VR * SW], bf16, kind="ExternalInput").ap()
    ksm_d = nc.dram_tensor("ksm", [VR, L], bf16, kind="ExternalInput").ap()
    scr_d = nc.dram_tensor("scr", [2, NT, VR * SW], bf16).ap()
    loss_d = nc.dram_tensor("loss", [NE, 1], f32, kind="ExternalOutput").ap()

    with tile.TileContext(nc) as tc, ExitStack() as ctx:
        const_p = ctx.enter_context(tc.tile_pool(name="const", bufs=1))
        state_p = ctx.enter_context(tc.tile_pool(name="state", bufs=1))
        chunk_p = ctx.enter_context(tc.tile_pool(name="chunk", bufs=2))
        xt_p = ctx.enter_context(tc.tile_pool(name="xt", bufs=2))
        evb_p = ctx.enter_context(tc.tile_pool(name="evb", bufs=2))
        ps_p = ctx.enter_context(
            tc.tile_pool(name="ps", bufs=4, space=bass.MemorySpace.PSUM)
        )
        ep_p = ctx.enter_context(tc.tile_pool(name="ep", bufs=1))

        V, S = nc.vector, nc.scalar

        # ---- constants
        ksm_t = const_p.tile([VR, L], bf16, tag="ksm")
        nc.sync.dma_start(ksm_t[:, :], ksm_d[:, :])
        sel_t = const_p.tile([128, 2, VR, SW], bf16, tag="sel")
        nc.sync.dma_start(
            sel_t[:, :, :, :],
            sel_d.rearrange("c (x v s) -> c x v s", x=2, s=SW),
        )

        # ---- state tiles (bf16), ping-pong pairs
        def zt(tag, w):
            t = state_p.tile([VR, w], bf16, tag=tag)
            V.memset(t[:, :], 0.0)
            return t

        gA, gB = zt("gA", L), zt("gB", L)
        DA, DB = zt("DA", 129), zt("DB", 129)
        hA, hB = zt("hA", 129), zt("hB", 129)      # col0 = permanent 0 guard
        kA, kB = zt("kA", 129), zt("kB", 129)      # col0 = permanent 0 guard
        tA, tB = zt("tA", L), zt("tB", L)
        utT = state_p.tile([VR, 129], bf16, tag="utT")
        V.memset(gA[:, 0:1], 1.0)
        V.memset(DA[:, 0:1], float(np.exp(CEQ)))

        rmax = state_p.tile([VR, NR], f32, tag="rmax")
        rinv = state_p.tile([VR, 1], f32, tag="rinv")

        cur_g, new_g = gA, gB
        cur_D, new_D = DA, DB
        cur_h, new_h = hA, hB
        cur_k, new_k = kA, kB
        cur_t, new_t = tA, tB

        # ---- main loop
        for blk in range(NCHUNK):
            xts = []
            for d, srcd in enumerate((xpf_d, xpb_d)):
                xb = xt_p.tile([128, 2 * NE, NT], bf16, tag=f"xb{d}")
                nc.sync.dma_start(
                    xb[:, :, :],
                    srcd.rearrange("e (cc c) t -> c (e cc) t", cc=2)[
                        :, :, blk * NT:(blk + 1) * NT
                    ],
                )
                xts.append(xb)
            scrb = scr_d[blk % 2].rearrange("m (v s) -> m v s", s=SW)
            for half in range(2):
                evb = evb_p.tile([NT, NE * SW], bf16, tag="evb")
                for vq in range(NE):
                    v = half * NE + vq
                    xb = xts[half]
                    ps = ps_p.tile([NT, SW], f32, tag="gps")
                    nc.tensor.matmul(
                        ps[:, :], xb[:, 2 * vq, :], sel_t[:, 0, v, :],
                        start=True, stop=False,
                    )
                    nc.tensor.matmul(
                        ps[:, :], xb[:, 2 * vq + 1, :], sel_t[:, 1, v, :],
                        start=False, stop=True,
                    )
                    S.copy(evb[:, vq * SW:(vq + 1) * SW], ps[:, :])
                nc.sync.dma_start(
                    scrb[:, half * NE:(half + 1) * NE, :], evb[:, :]
                )
            plc = chunk_p.tile([VR, NT * SW], bf16, tag="plc")
            nc.sync.dma_start(
                plc[:, :],
                scr_d[blk % 2].rearrange("m (v s) -> v m s", s=SW),
            )
            # scalar operands must be fp32: extract the two blank columns
            plc3 = plc[:, :].rearrange("v (m s) -> v m s", s=SW)
            pbf = chunk_p.tile([VR, NT, 2], f32, tag="pbf")
            S.copy(pbf[:, :, :], plc3[:, :, 128:130])

            for ml in range(NT):
                m = blk * NT + ml
                pl = plc[:, ml * SW: ml * SW + 128]
                pb = pbf[:, ml, 0:1]
                pb2 = pbf[:, ml, 1:2]
                g0, D0 = cur_g, cur_D
                hN, kN, tN = new_h, new_k, new_t
                g1, D1 = new_g, new_D

                V.tensor_tensor(hN[:, 1:129], g0[:, :], pl, Alu.mult)
                V.scalar_tensor_tensor(
                    tN[:, :], D0[:, 0:128], pb2, hN[:, 1:129], Alu.mult, Alu.add
                )
                V.scalar_tensor_tensor(
                    D1[:, :], D0[:, :], pb, hN[:, 0:129], Alu.mult, Alu.add
                )
                V.tensor_tensor(kN[:, 1:129], hN[:, 1:129], ksm_t[:, :], Alu.mult)
                V.tensor_tensor(g1[:, :], tN[:, :], kN[:, 0:128], Alu.add)

                if m == TH - 1:
                    V.tensor_single_scalar(utT[:, :], D0[:, :], pb, Alu.mult)

                if (m + 1) % RB == 0:
                    r = (m + 1) // RB - 1
                    V.tensor_reduce(
                        rmax[:, r:r + 1], D1[:, :],
                        axis=mybir.AxisListType.X, op=Alu.max,
                    )
                    V.tensor_single_scalar(
                        rmax[:, r:r + 1], rmax[:, r:r + 1], 1e-30, Alu.max
                    )
                    V.reciprocal(rinv[:, :], rmax[:, r:r + 1])
                    V.tensor_single_scalar(g1[:, :], g1[:, :], rinv[:, 0:1], Alu.mult)
                    V.tensor_single_scalar(D1[:, :], D1[:, :], rinv[:, 0:1], Alu.mult)
                    if m == TH - 1:
                        V.tensor_single_scalar(
                            hN[:, 1:129], hN[:, 1:129], rinv[:, 0:1], Alu.mult
                        )
                        V.tensor_single_scalar(
                            utT[:, :], utT[:, :], rinv[:, 0:1], Alu.mult
                        )

                cur_g, new_g = new_g, cur_g
                cur_D, new_D = new_D, cur_D
                cur_h, new_h = new_h, cur_h
                cur_k, new_k = new_k, cur_k
                cur_t, new_t = new_t, cur_t

        # ---- epilogue: log-space combine
        # fwd rows (0:NE): aL = h_final, aB = ut_final
        # bwd rows (NE:VR): bL = g_final, bB = D_final
        hF = cur_h    # last written h
        gF, DF = cur_g, cur_D

        hf32 = ep_p.tile([VR, L], f32, tag="hf32")
        uf32 = ep_p.tile([VR, 129], f32, tag="uf32")
        gf32 = ep_p.tile([VR, L], f32, tag="gf32")
        Df32 = ep_p.tile([VR, 129], f32, tag="Df32")
        S.copy(hf32[:, :], hF[:, 1:129])
        S.copy(uf32[:, :], utT[:, :])
        S.copy(gf32[:, :], gF[:, :])
        S.copy(Df32[:, :], DF[:, :])

        sln_i = ep_p.tile([VR, 129], i32, tag="sln_i")
        sln_m = ep_p.tile([VR, 129], i32, tag="sln_m")
        sln_e = ep_p.tile([VR, 129], f32, tag="sln_e")
        sln_l = ep_p.tile([VR, 129], f32, tag="sln_l")

        def safe_ln(dst_ap, src_ap, n):
            """dst = ln(src) via exponent extraction; HW Ln table is only
            accurate on ~[1e-10, 2^64]; mantissa lives in [1, 2)."""
            ii = sln_i[:, 0:n]
            mm = sln_m[:, 0:n]
            ee = sln_e[:, 0:n]
            ll = sln_l[:, 0:n]
            V.tensor_single_scalar(ii, src_ap.bitcast(i32), 23, Alu.arith_shift_right)
            V.tensor_single_scalar(ii, ii, 127, Alu.subtract)
            V.tensor_single_scalar(mm, src_ap.bitcast(i32), 0x007FFFFF, Alu.bitwise_and)
            V.tensor_single_scalar(mm, mm, 0x3F800000, Alu.bitwise_or)
            S.activation(ll, mm.bitcast(f32), Act.Ln)
            V.tensor_copy(ee, ii)
            V.scalar_tensor_tensor(
                dst_ap, ee, 0.6931471805599453, ll, Alu.mult, Alu.add
            )

        # lZ = sum of ln(rmax) over the 16 rescales
        lnr = ep_p.tile([VR, NR], f32, tag="lnr")
        lZ = ep_p.tile([VR, 1], f32, tag="lZ")
        safe_ln(lnr[:, :], rmax[:, :], NR)
        V.tensor_reduce(lZ[:, :], lnr[:, :], axis=mybir.AxisListType.X, op=Alu.add)

        LL = ep_p.tile([VR, L], f32, tag="LL")
        LB = ep_p.tile([VR, 129], f32, tag="LB")
        cl1 = ep_p.tile([VR, L], f32, tag="cl1")
        eq1 = ep_p.tile([VR, L], f32, tag="eq1")
        ln1 = ep_p.tile([VR, L], f32, tag="ln1")
        cl2 = ep_p.tile([VR, 129], f32, tag="cl2")
        eq2 = ep_p.tile([VR, 129], f32, tag="eq2")
        ln2 = ep_p.tile([VR, 129], f32, tag="ln2")

        V.tensor_single_scalar(cl1[0:NE, :], hf32[0:NE, :], TINY, Alu.max)
        V.tensor_single_scalar(cl1[NE:VR, :], gf32[NE:VR, :], TINY, Alu.max)
        V.tensor_single_scalar(eq1[0:NE, :], hf32[0:NE, :], 0.0, Alu.is_equal)
        V.tensor_single_scalar(eq1[NE:VR, :], gf32[NE:VR, :], 0.0, Alu.is_equal)
        safe_ln(ln1[:, :], cl1[:, :], L)
        V.scalar_tensor_tensor(LL[:, :], eq1[:, :], NEG, ln1[:, :], Alu.mult, Alu.add)

        V.tensor_single_scalar(cl2[0:NE, :], uf32[0:NE, :], TINY, Alu.max)
        V.tensor_single_scalar(cl2[NE:VR, :], Df32[NE:VR, :], TINY, Alu.max)
        V.tensor_single_scalar(eq2[0:NE, :], uf32[0:NE, :], 0.0, Alu.is_equal)
        V.tensor_single_scalar(eq2[NE:VR, :], Df32[NE:VR, :], 0.0, Alu.is_equal)
        safe_ln(ln2[:, :], cl2[:, :], 129)
        V.scalar_tensor_tensor(LB[:, :], eq2[:, :], NEG, ln2[:, :], Alu.mult, Alu.add)

        V.tensor_single_scalar(LL[:, :], LL[:, :], lZ[:, 0:1], Alu.add)
        V.tensor_single_scalar(LB[:, :], LB[:, :], lZ[:, 0:1], Alu.add)

        LLb = ep_p.tile([NE, L], f32, tag="LLb")
        LBb = ep_p.tile([NE, 129], f32, tag="LBb")
        nc.sync.dma_start(LLb[:, :], LL[NE:VR, ::-1])
        nc.sync.dma_start(LBb[:, :], LB[NE:VR, ::-1])

        lPL = ep_p.tile([NE, L], f32, tag="lPL")
        lPB = ep_p.tile([NE, 129], f32, tag="lPB")
        V.tensor_tensor(lPL[:, :], LL[0:NE, :], LLb[:, :], Alu.add)
        V.tensor_tensor(lPB[:, :], LB[0:NE, :], LBb[:, :], Alu.add)
        # equilibration pairing constants
        V.tensor_single_scalar(lPL[:, :], lPL[:, :], 127.0 * CEQ, Alu.add)
        V.tensor_single_scalar(lPB[:, :], lPB[:, :], 126.0 * CEQ, Alu.add)

        m1 = ep_p.tile([NE, 1], f32, tag="m1")
        m2 = ep_p.tile([NE, 1], f32, tag="m2")
        V.tensor_reduce(m1[:, :], lPL[:, :], axis=mybir.AxisListType.X, op=Alu.max)
        V.tensor_reduce(m2[:, :], lPB[:, :], axis=mybir.AxisListType.X, op=Alu.max)
        V.tensor_tensor(m1[:, :], m1[:, :], m2[:, :], Alu.max)
        sm1 = ep_p.tile([NE, L], f32, tag="sm1")
        sm2 = ep_p.tile([NE, 129], f32, tag="sm2")
        V.tensor_single_scalar(sm1[:, :], lPL[:, :], m1[:, 0:1], Alu.subtract)
        V.tensor_single_scalar(sm2[:, :], lPB[:, :], m1[:, 0:1], Alu.subtract)
        e1 = ep_p.tile([NE, L], f32, tag="e1")
        e2 = ep_p.tile([NE, 129], f32, tag="e2")
        s1 = ep_p.tile([NE, 1], f32, tag="s1")
        s2 = ep_p.tile([NE, 1], f32, tag="s2")
        S.activation(e1[:, :], sm1[:, :], Act.Exp, accum_out=s1[:, :])
        S.activation(e2[:, :], sm2[:, :], Act.Exp, accum_out=s2[:, :])
        V.tensor_tensor(s1[:, :], s1[:, :], s2[:, :], Alu.add)
        lsum = ep_p.tile([NE, 1], f32, tag="lsum")
        S.activation(lsum[:, :], s1[:, :], Act.Ln)
        lossT = ep_p.tile([NE, 1], f32, tag="lossT")
        V.tensor_tensor(lossT[:, :], m1[:, :], lsum[:, :], Alu.add)
        V.tensor_single_scalar(lossT[:, :], lossT[:, :], -1.0, Alu.mult)
        nc.sync.dma_start(loss_d[:, :], lossT[:, :])

    nc.compile()
    return nc


# ------------------------------- host side ------------------------------------

def make_inputs(y_true, y_pred):
    """Build the 8 per-core input maps (all bf16)."""
    y_true = np.asarray(y_true)
    y_pred = np.asarray(y_pred, dtype=np.float32)
    p = (y_pred + np.float32(EPS)).astype(BF16NP)          # [B, T, C]

    lab = y_true.astype(np.int64)                          # [B, L]
    k = np.ones((B, L), np.float32)
    k[:, 1:] = (lab[:, 1:] != lab[:, :-1]).astype(np.float32)
    ks = np.ones((B, L), np.float32)
    ks[:, :-1] = k[:, 1:]
    e_c = np.float32(np.exp(-CEQ))

    cgrid = np.arange(128, dtype=np.int64)
    w = np.ones(SW, np.float32)
    w[129] = e_c

    in_maps = []
    for q in range(NCORES):
        sl = slice(q * NE, (q + 1) * NE)
        pq = p[sl]                                          # [NE, T, C]
        xpf = np.ascontiguousarray(pq[:, :TH, :].transpose(0, 2, 1))
        xpb = np.ascontiguousarray(pq[:, TH:, :][:, ::-1, :].transpose(0, 2, 1))

        labext = np.full((VR, SW), C - 1, np.int64)         # cols 128,129 = blank
        labext[:NE, :L] = lab[sl]
        labext[NE:, :L] = lab[sl][:, ::-1]
        # sel[c', cc, v, s] = (labext[v,s] == c' + 128*cc) * w[s]
        sel = (
            labext[None, None, :, :] == (cgrid[:, None, None, None] + 128 * np.arange(2)[None, :, None, None])
        ).astype(np.float32) * w[None, None, None, :]
        sel = np.ascontiguousarray(sel.reshape(128, 2 * VR * SW).astype(BF16NP))

        ksm = np.empty((VR, L), np.float32)
        ksm[:NE] = ks[sl]
        ksm[NE:] = k[sl][:, ::-1]
        ksm = (ksm * e_c).astype(BF16NP)

        in_maps.append({
            "xpf": xpf,
            "xpb": xpb,
            "sel": sel,
            "ksm": ksm,
        })
    return in_maps


_NC_CACHE = {}


def _get_nc():
    if "nc" not in _NC_CACHE:
        _NC_CACHE["nc"] = build_nc()
    return _NC_CACHE["nc"]


def kernel(y_true, y_pred):
    _ensure_path()
    from concourse.bass_utils import run_bass_kernel_spmd

    nc = _get_nc()
    in_maps = make_inputs(y_true, y_pred)
    res = run_bass_kernel_spmd(nc, in_maps, core_ids=list(range(NCORES)))
    loss = np.concatenate([r["loss"] for r in res.results], axis=0)
    return loss.astype(np.float32)


if __name__ == "__main__":
    nc = build_nc()
    print("built + compiled OK")
